# revision 35
# baseline (speedup 1.0000x reference)
"""Trainium2 Bass kernel for nn_ATTNLoss (top-k masked attention reconstruction loss).

Math: loss = mean((x-y)^2) + ALPHA * mean((attn - topk32(attn))^2)
Since topk scattering only zeroes the top-32 entries of each row:
    attn_loss = (sum(attn^2) - sum_{rows} sum(top32(row)^2)) / N^2
so nothing sparse needs materializing; only three scalar sums are needed.

Sharding: rows split evenly across 8 NeuronCores (top-k is row-local).
Each core computes per-partition partial sums; the host combines them in
float64 and forms the final scalar.

Per-row top-32 on device: per-row top-8 of each column block (nc.vector.max)
produces a candidate set; 4 rounds of max+match_replace on the narrow
candidate buffer yield the exact top-32 values provided no block holds >8 of
the row's top-32 elements. kernel() verifies that property on the actual
input on the host (cheap numpy check) and falls back to a smaller block size
or a full-width exact variant if ever violated.
"""

import numpy as np

N = 8192  # attention matrix is [N, N]
D = 1024  # reconstruction feature dim
K = 32  # top-k
ALPHA = 0.1
N_CORES = 8
ROWS = N // N_CORES  # rows per core = 1024
P = 128  # SBUF partitions
NT = ROWS // P  # row-tiles per core = 8

_BUILDS: dict = {}


def _build_bass(blk: int, head_chunks=(1024, 3072, 4096)):
    """Build the per-core Bass module.

    blk > 0: level-1 block size for the block-top8 candidate pass.
    blk == 0: exact full-width fallback (4 rounds of max+match_replace over
    the whole 8192-wide row).
    head_chunks: column split of the first row-tile; a small leading chunk
    lets the vector engine start before the whole 4MB tile lands.
    """
    import concourse.tile as tile
    from concourse import bacc, mybir
    from concourse.tile_rust import add_dep_helper

    f32 = mybir.dt.float32
    Sq = mybir.ActivationFunctionType.Square
    AX = mybir.AxisListType.X
    ADD = mybir.AluOpType.add

    # Bacc (not raw Bass): its compile() pass splits multi-wait sync_infos,
    # which the TRN2 ISA requires (at most one wait per instruction).
    nc = bacc.Bacc()
    attn_in = nc.declare_dram_parameter("attn", [ROWS, N], f32, isOutput=False)
    # host passes x and NEGATED y; an SWDGE accumulate-add DMA computes
    # x + (-y) inline in the SDMA datapath, so no engine does the subtract.
    x_in = nc.declare_dram_parameter("x", [ROWS, D], f32, isOutput=False)
    yneg_in = nc.declare_dram_parameter("yneg", [ROWS, D], f32, isOutput=False)
    out_ext = nc.declare_dram_parameter("out", [P, 4], f32, isOutput=True)

    with tile.TileContext(nc) as tc:
        with (
            tc.tile_pool(name="attn_p", bufs=3) as attn_p,
            tc.tile_pool(name="attn0_p", bufs=1) as attn0_p,
            tc.tile_pool(name="attn7_p", bufs=1) as attn7_p,
            tc.tile_pool(name="xy_p", bufs=6) as xy_p,
            tc.tile_pool(name="small_p", bufs=2) as small_p,
            tc.tile_pool(name="acc_p", bufs=1) as acc_p,
        ):
            # acc columns: [0:NT) sum(x-y)^2, [NT:2NT) sum(attn^2) pieces,
            # [2NT:3NT) sum(top32^2) pieces, [3NT:4NT) extra sum(attn^2)
            # pieces from split tiles (main) / head-24 top^2 (fallback).
            acc = acc_p.tile([P, 4 * NT], f32)
            nc.vector.memset(acc[:], 0.0)

            extra_col = 3 * NT  # next free "extra attn^2" accumulator column
            last_dve = None
            last_attn_dma = None
            attn_dmas = []  # last DMA of each row-tile, for xy staggering
            for t in range(NT):
                top = small_p.tile([P, K], f32, tag="top")
                if blk > 0:
                    nb = N // blk
                    cw = nb * 8
                    chunks = []
                    if t == 0 and head_chunks:
                        widths = list(head_chunks)
                    elif t == NT - 1:
                        widths = [N // 2, N // 2]
                    else:
                        widths = [N]
                    if len(widths) > 1:
                        c0 = 0
                        pool = attn0_p if t == 0 else attn7_p
                        for ci, cwid in enumerate(widths):
                            ct = pool.tile([P, cwid], f32, tag=f"a{t}_{ci}")
                            last_attn_dma = nc.sync.dma_start(
                                out=ct[:],
                                in_=attn_in[t * P : (t + 1) * P, c0 : c0 + cwid],
                            )
                            chunks.append((ct, c0, cwid))
                            c0 += cwid
                    else:
                        a = attn_p.tile([P, N], f32, tag="a")
                        last_attn_dma = nc.sync.dma_start(
                            out=a[:], in_=attn_in[t * P : (t + 1) * P, :]
                        )
                        chunks = [(a, 0, N)]
                    attn_dmas.append(last_attn_dma)

                    cand = small_p.tile([P, cw], f32, tag="cand")
                    for ct, c0_, cwid in chunks:
                        for b in range(cwid // blk):
                            g = c0_ // blk + b  # global block index
                            nc.vector.max(
                                out=cand[:, g * 8 : (g + 1) * 8],
                                in_=ct[:, b * blk : (b + 1) * blk],
                            )
                    for r in range(K // 8):
                        last_dve = nc.vector.max(
                            out=top[:, r * 8 : (r + 1) * 8], in_=cand[:]
                        )
                        if r < K // 8 - 1:
                            nc.vector.match_replace(
                                out=cand[:],
                                in_to_replace=top[:, r * 8 : (r + 1) * 8],
                                in_values=cand[:],
                                imm_value=0.0,
                            )
                    # sum(top32^2) for this tile
                    nc.scalar.activation(
                        out=top[:], in_=top[:], func=Sq,
                        accum_out=acc[:, 2 * NT + t : 2 * NT + t + 1],
                    )
                    # sum(attn^2) (in-place square; the chunk is dead after)
                    for ci, (ct, c0_, cwid) in enumerate(chunks):
                        if ci == 0:
                            col = NT + t
                        else:
                            col = extra_col
                            extra_col += 1
                        nc.scalar.activation(
                            out=ct[:], in_=ct[:], func=Sq,
                            accum_out=acc[:, col : col + 1],
                        )
                else:
                    # Exact fallback: extract top-32 directly from the full row.
                    # match_replace zeroes the extracted values in `a`, so
                    # sum(attn^2) = sum(a_modified^2) + sum(top24_extracted^2).
                    a = attn_p.tile([P, N], f32, tag="a")
                    last_attn_dma = nc.sync.dma_start(
                        out=a[:], in_=attn_in[t * P : (t + 1) * P, :]
                    )
                    attn_dmas.append(last_attn_dma)
                    for r in range(K // 8):
                        last_dve = nc.vector.max(out=top[:, r * 8 : (r + 1) * 8], in_=a[:])
                        if r < K // 8 - 1:
                            nc.vector.match_replace(
                                out=a[:],
                                in_to_replace=top[:, r * 8 : (r + 1) * 8],
                                in_values=a[:],
                                imm_value=0.0,
                            )
                    # first 24 values were zeroed out of `a`
                    nc.scalar.activation(
                        out=top[:, : K - 8], in_=top[:, : K - 8], func=Sq,
                        accum_out=acc[:, 3 * NT + t : 3 * NT + t + 1],
                    )
                    # last 8 remain in `a`
                    nc.scalar.activation(
                        out=top[:, K - 8 :], in_=top[:, K - 8 :], func=Sq,
                        accum_out=acc[:, 2 * NT + t : 2 * NT + t + 1],
                    )
                    nc.scalar.activation(
                        out=a[:], in_=a[:], func=Sq,
                        accum_out=acc[:, NT + t : NT + t + 1],
                    )

            # xy work emitted after the attn loop; each x DMA is staggered
            # behind the NEXT tile's attn DMA so its 1MB rides the per-tile
            # bandwidth slack without starving the vector engine, and the xy
            # squares complete mid-kernel instead of serializing at the tail.
            for t in range(NT):
                xt = xy_p.tile([P, D], f32, tag="xt")
                xdma = nc.sync.dma_start(out=xt[:], in_=x_in[t * P : (t + 1) * P, :])
                anchor = attn_dmas[min(t + 1, len(attn_dmas) - 1)]
                add_dep_helper(
                    xdma.ins, anchor.ins, sync=False,
                    reason="xy DMA trails the next attn tile's DMA",
                )
                nc.gpsimd.dma_start(
                    out=xt[:], in_=yneg_in[t * P : (t + 1) * P, :],
                    accum_op=mybir.AluOpType.add,
                )
                nc.scalar.activation(
                    out=xt[:], in_=xt[:], func=Sq, accum_out=acc[:, t : t + 1]
                )

            osb = acc_p.tile([P, 4], f32)
            for c in range(4):
                r = nc.vector.tensor_reduce(
                    out=osb[:, c : c + 1],
                    in_=acc[:, c * NT : (c + 1) * NT],
                    axis=AX,
                    op=ADD,
                )
                # pin after the last hot DVE op: the scheduler otherwise may
                # park a reduce mid-queue and stall the in-order DVE engine.
                add_dep_helper(
                    r.ins, last_dve.ins, sync=False,
                    reason="final reduces run after the last top-k op",
                )
            nc.sync.dma_start(out=out_ext[:], in_=osb[:])

    nc.finalize()  # runs Bacc.compile(): wait splitting + register allocation
    return nc


def _get_nc(blk: int):
    if blk not in _BUILDS:
        _BUILDS[blk] = _build_bass(blk)
    return _BUILDS[blk]


def _pick_blk(attn: np.ndarray) -> int:
    """Choose the largest safe level-1 block size for this input.

    Safe means: for every row, no block contains more than 8 elements that
    are >= the row's 32nd-largest value (so block-top8 candidates provably
    contain every valid top-32 choice).
    """
    t32 = np.partition(attn, N - K, axis=1)[:, N - K]
    ge = attn >= t32[:, None]
    for blk in (256, 128):
        nb = N // blk
        cnt = ge.reshape(N, nb, blk).sum(axis=2, dtype=np.int32)
        if cnt.max() <= 8:
            return blk
    return 0


def _combine(results, blk: int) -> np.float32:
    S = np.zeros(4, dtype=np.float64)
    for r in results:
        S += r["out"].astype(np.float64).sum(axis=0)
    sxy, sattn, s2, s3 = S
    # main path: col3 = extra sum(attn^2) pieces, col2 = full top32^2.
    # fallback: col3 = head-24 top^2 (also missing from col1's sum(attn^2)
    # because match_replace zeroed those entries), col2 = tail-8 top^2.
    sattn = sattn + s3
    stop = s2 if blk > 0 else s2 + s3
    loss = sxy / (N * D) + ALPHA * (sattn - stop) / (N * N)
    return np.float32(loss)


def _shard(x: np.ndarray, y: np.ndarray, attn: np.ndarray):
    in_maps = []
    for c in range(N_CORES):
        r0, r1 = c * ROWS, (c + 1) * ROWS
        in_maps.append(
            {
                "attn": np.ascontiguousarray(attn[r0:r1]),
                "x": np.ascontiguousarray(x[r0:r1]),
                "yneg": -y[r0:r1],
            }
        )
    return in_maps


def kernel(x: np.ndarray, y: np.ndarray, attn: np.ndarray) -> np.ndarray:
    from concourse.bass_utils import run_bass_kernel_spmd

    x = np.asarray(x, dtype=np.float32)
    y = np.asarray(y, dtype=np.float32)
    attn = np.asarray(attn, dtype=np.float32)

    blk = _pick_blk(attn)
    nc = _get_nc(blk)
    res = run_bass_kernel_spmd(nc, _shard(x, y, attn), list(range(N_CORES)))
    return np.asarray(_combine(res.results, blk))


# revision 39
# speedup vs baseline: 1.1938x; 1.1938x over previous
"""Trainium2 Bass kernel for nn_ATTNLoss (top-k masked attention reconstruction loss).

Math: loss = mean((x-y)^2) + ALPHA * mean((attn - topk32(attn))^2)
Since topk scattering only zeroes the top-32 entries of each row:
    attn_loss = (sum(attn^2) - sum_{rows} sum(top32(row)^2)) / N^2
so nothing sparse needs materializing; only three scalar sums are needed.

Sharding: rows split evenly across 8 NeuronCores (top-k is row-local).
Each core computes per-partition partial sums; the host combines them in
float64 and forms the final scalar.

Per-row top-32 on device: per-row top-8 of each column block (nc.vector.max)
produces a candidate set; 4 rounds of max+match_replace on the narrow
candidate buffer yield the exact top-32 values provided no block holds >8 of
the row's top-32 elements. kernel() verifies that property on the actual
input on the host (cheap numpy check) and falls back to a smaller block size
or a full-width exact variant if ever violated.
"""

import numpy as np

N = 8192  # attention matrix is [N, N]
D = 1024  # reconstruction feature dim
K = 32  # top-k
ALPHA = 0.1
N_CORES = 8
ROWS = N // N_CORES  # rows per core = 1024
P = 128  # SBUF partitions
NT = ROWS // P  # row-tiles per core = 8

_BUILDS: dict = {}


def _build_bass(blk: int, head_chunks=(1024, 3072, 4096)):
    """Build the per-core Bass module.

    blk > 0: level-1 block size for the block-top8 candidate pass.
    blk == 0: exact full-width fallback (4 rounds of max+match_replace over
    the whole 8192-wide row).
    head_chunks: column split of the first row-tile; a small leading chunk
    lets the vector engine start before the whole 4MB tile lands.
    """
    import concourse.tile as tile
    from concourse import bacc, mybir
    from concourse.tile_rust import add_dep_helper

    f32 = mybir.dt.float32
    Sq = mybir.ActivationFunctionType.Square
    AX = mybir.AxisListType.X
    ADD = mybir.AluOpType.add

    # Bacc (not raw Bass): its compile() pass splits multi-wait sync_infos,
    # which the TRN2 ISA requires (at most one wait per instruction).
    nc = bacc.Bacc()
    attn_in = nc.declare_dram_parameter("attn", [ROWS, N], f32, isOutput=False)
    # host passes x and NEGATED y; an SWDGE accumulate-add DMA computes
    # x + (-y) inline in the SDMA datapath, so no engine does the subtract.
    x_in = nc.declare_dram_parameter("x", [ROWS, D], f32, isOutput=False)
    yneg_in = nc.declare_dram_parameter("yneg", [ROWS, D], f32, isOutput=False)
    out_ext = nc.declare_dram_parameter("out", [P, 4], f32, isOutput=True)

    with tile.TileContext(nc) as tc:
        with (
            tc.tile_pool(name="attn_p", bufs=4) as attn_p,
            tc.tile_pool(name="attn0_p", bufs=1) as attn0_p,
            tc.tile_pool(name="attn7_p", bufs=1) as attn7_p,
            tc.tile_pool(name="xy_p", bufs=NT) as xy_p,
            tc.tile_pool(name="small_p", bufs=2) as small_p,
            tc.tile_pool(name="acc_p", bufs=1) as acc_p,
        ):
            # acc columns: [0:NT) sum(x-y)^2, [NT:2NT) sum(attn^2) pieces,
            # [2NT:3NT) sum(top32^2) pieces, [3NT:4NT) extra sum(attn^2)
            # pieces from split tiles (main) / head-24 top^2 (fallback).
            acc = acc_p.tile([P, 4 * NT], f32)
            nc.vector.memset(acc[:], 0.0)

            extra_col = 3 * NT  # next free "extra attn^2" accumulator column
            last_dve = None
            last_attn_dma = None
            attn_dmas = []  # last DMA of each row-tile, for xy staggering
            for t in range(NT):
                top = small_p.tile([P, K], f32, tag="top")
                if blk > 0:
                    nb = N // blk
                    cw = nb * 8
                    chunks = []
                    if t == 0 and head_chunks:
                        widths = list(head_chunks)
                    elif t == NT - 1:
                        widths = [N // 2, N // 2]
                    else:
                        widths = [N]
                    if len(widths) > 1:
                        c0 = 0
                        for ci, cwid in enumerate(widths):
                            # the 4096-wide chunks ride in the main rotating
                            # pool (slots are sized for 8192 anyway) so the
                            # dedicated boundary pools stay small.
                            if cwid > 3072:
                                ct = attn_p.tile([P, cwid], f32, tag="a")
                            else:
                                pool = attn0_p if t == 0 else attn7_p
                                ct = pool.tile([P, cwid], f32, tag=f"a{t}_{ci}")
                            last_attn_dma = nc.sync.dma_start(
                                out=ct[:],
                                in_=attn_in[t * P : (t + 1) * P, c0 : c0 + cwid],
                            )
                            chunks.append((ct, c0, cwid))
                            c0 += cwid
                    else:
                        a = attn_p.tile([P, N], f32, tag="a")
                        last_attn_dma = nc.sync.dma_start(
                            out=a[:], in_=attn_in[t * P : (t + 1) * P, :]
                        )
                        chunks = [(a, 0, N)]
                    attn_dmas.append(last_attn_dma)

                    cand = small_p.tile([P, cw], f32, tag="cand")
                    for ct, c0_, cwid in chunks:
                        for b in range(cwid // blk):
                            g = c0_ // blk + b  # global block index
                            nc.vector.max(
                                out=cand[:, g * 8 : (g + 1) * 8],
                                in_=ct[:, b * blk : (b + 1) * blk],
                            )
                    for r in range(K // 8):
                        last_dve = nc.vector.max(
                            out=top[:, r * 8 : (r + 1) * 8], in_=cand[:]
                        )
                        if r < K // 8 - 1:
                            nc.vector.match_replace(
                                out=cand[:],
                                in_to_replace=top[:, r * 8 : (r + 1) * 8],
                                in_values=cand[:],
                                imm_value=0.0,
                            )
                    # sum(top32^2) for this tile
                    nc.scalar.activation(
                        out=top[:], in_=top[:], func=Sq,
                        accum_out=acc[:, 2 * NT + t : 2 * NT + t + 1],
                    )
                    # sum(attn^2) (in-place square; the chunk is dead after)
                    for ci, (ct, c0_, cwid) in enumerate(chunks):
                        if ci == 0:
                            col = NT + t
                        else:
                            col = extra_col
                            extra_col += 1
                        nc.scalar.activation(
                            out=ct[:], in_=ct[:], func=Sq,
                            accum_out=acc[:, col : col + 1],
                        )
                else:
                    # Exact fallback: extract top-32 directly from the full row.
                    # match_replace zeroes the extracted values in `a`, so
                    # sum(attn^2) = sum(a_modified^2) + sum(top24_extracted^2).
                    a = attn_p.tile([P, N], f32, tag="a")
                    last_attn_dma = nc.sync.dma_start(
                        out=a[:], in_=attn_in[t * P : (t + 1) * P, :]
                    )
                    attn_dmas.append(last_attn_dma)
                    for r in range(K // 8):
                        last_dve = nc.vector.max(out=top[:, r * 8 : (r + 1) * 8], in_=a[:])
                        if r < K // 8 - 1:
                            nc.vector.match_replace(
                                out=a[:],
                                in_to_replace=top[:, r * 8 : (r + 1) * 8],
                                in_values=a[:],
                                imm_value=0.0,
                            )
                    # first 24 values were zeroed out of `a`
                    nc.scalar.activation(
                        out=top[:, : K - 8], in_=top[:, : K - 8], func=Sq,
                        accum_out=acc[:, 3 * NT + t : 3 * NT + t + 1],
                    )
                    # last 8 remain in `a`
                    nc.scalar.activation(
                        out=top[:, K - 8 :], in_=top[:, K - 8 :], func=Sq,
                        accum_out=acc[:, 2 * NT + t : 2 * NT + t + 1],
                    )
                    nc.scalar.activation(
                        out=a[:], in_=a[:], func=Sq,
                        accum_out=acc[:, NT + t : NT + t + 1],
                    )

            # xy work LAST: there is no spare DMA bandwidth mid-kernel (the
            # attn stream runs neck-and-neck with the vector engine), so the
            # 8MB of xy bytes ride at the end where only ACT still needs them.
            for t in range(NT):
                xt = xy_p.tile([P, D], f32, tag="xt")
                xdma = nc.sync.dma_start(out=xt[:], in_=x_in[t * P : (t + 1) * P, :])
                add_dep_helper(
                    xdma.ins, attn_dmas[-1].ins, sync=False,
                    reason="xy DMAs trail the attn stream",
                )
                nc.gpsimd.dma_start(
                    out=xt[:], in_=yneg_in[t * P : (t + 1) * P, :],
                    accum_op=mybir.AluOpType.add,
                )
                nc.scalar.activation(
                    out=xt[:], in_=xt[:], func=Sq, accum_out=acc[:, t : t + 1]
                )

            osb = acc_p.tile([P, 4], f32)
            for c in range(4):
                r = nc.vector.tensor_reduce(
                    out=osb[:, c : c + 1],
                    in_=acc[:, c * NT : (c + 1) * NT],
                    axis=AX,
                    op=ADD,
                )
                # pin after the last hot DVE op: the scheduler otherwise may
                # park a reduce mid-queue and stall the in-order DVE engine.
                add_dep_helper(
                    r.ins, last_dve.ins, sync=False,
                    reason="final reduces run after the last top-k op",
                )
            nc.sync.dma_start(out=out_ext[:], in_=osb[:])

    nc.finalize()  # runs Bacc.compile(): wait splitting + register allocation
    return nc


def _get_nc(blk: int):
    if blk not in _BUILDS:
        _BUILDS[blk] = _build_bass(blk)
    return _BUILDS[blk]


def _pick_blk(attn: np.ndarray) -> int:
    """Choose the largest safe level-1 block size for this input.

    Safe means: for every row, no block contains more than 8 elements that
    are >= the row's 32nd-largest value (so block-top8 candidates provably
    contain every valid top-32 choice).
    """
    t32 = np.partition(attn, N - K, axis=1)[:, N - K]
    ge = attn >= t32[:, None]
    for blk in (256, 128):
        nb = N // blk
        cnt = ge.reshape(N, nb, blk).sum(axis=2, dtype=np.int32)
        if cnt.max() <= 8:
            return blk
    return 0


def _combine(results, blk: int) -> np.float32:
    S = np.zeros(4, dtype=np.float64)
    for r in results:
        S += r["out"].astype(np.float64).sum(axis=0)
    sxy, sattn, s2, s3 = S
    # main path: col3 = extra sum(attn^2) pieces, col2 = full top32^2.
    # fallback: col3 = head-24 top^2 (also missing from col1's sum(attn^2)
    # because match_replace zeroed those entries), col2 = tail-8 top^2.
    sattn = sattn + s3
    stop = s2 if blk > 0 else s2 + s3
    loss = sxy / (N * D) + ALPHA * (sattn - stop) / (N * N)
    return np.float32(loss)


def _shard(x: np.ndarray, y: np.ndarray, attn: np.ndarray):
    in_maps = []
    for c in range(N_CORES):
        r0, r1 = c * ROWS, (c + 1) * ROWS
        in_maps.append(
            {
                "attn": np.ascontiguousarray(attn[r0:r1]),
                "x": np.ascontiguousarray(x[r0:r1]),
                "yneg": -y[r0:r1],
            }
        )
    return in_maps


def kernel(x: np.ndarray, y: np.ndarray, attn: np.ndarray) -> np.ndarray:
    from concourse.bass_utils import run_bass_kernel_spmd

    x = np.asarray(x, dtype=np.float32)
    y = np.asarray(y, dtype=np.float32)
    attn = np.asarray(attn, dtype=np.float32)

    blk = _pick_blk(attn)
    nc = _get_nc(blk)
    res = run_bass_kernel_spmd(nc, _shard(x, y, attn), list(range(N_CORES)))
    return np.asarray(_combine(res.results, blk))
